# revision 4
# baseline (speedup 1.0000x reference)
"""Multi-head attention (B=4, S=2048, D=1024, H=16) on 8 trn2 NeuronCores.

Sharding: data-parallel over batch (4) x tensor-parallel over heads (2 groups
of 8 heads), Megatron-style. Each core computes, for its (batch, head-group):
  Q^T = Wq_g x^T + bq_g        [512, 2048]  (head dim on partitions)
  K^T = Wk_g x^T               (bk dropped: softmax-invariant)
  V   = x Wv_g^T               [2048, 512]  (bv folded into host-side bias)
  S^T = K^T' Q^T / 8           per head [2048, 2048], no max-subtraction
  P^T = exp(S^T)  (bf16), r = sum_k P^T   (ones-matmul)
  U^T = V^T P^T   (col-tiled per head pair)
  A^T = U^T / r   -> y_partial = A Wo_g^T  [2048, 1024]  (f32)
Host: out[b] = y_partial[b,g=0] + y_partial[b,g=1] + (bo + bv @ Wo^T).
"""

import numpy as np
import ml_dtypes

import concourse.bass as bass
import concourse.tile as tile
from concourse import bacc, mybir
from concourse.bass import ts, ds
from concourse.bass_utils import run_bass_kernel_spmd

bf16 = ml_dtypes.bfloat16
F32, BF16 = mybir.dt.float32, mybir.dt.bfloat16
AF = mybir.ActivationFunctionType

B, S, D, H = 4, 2048, 1024, 16
DK = D // H          # 64
G = 2                # tensor-parallel groups
DG = D // G          # 512 dims per group
HP = H // G // 2     # head pairs per core = 4
KT = S // 128        # 16 k tiles
QC = S // 512        # 4 q chunks
P = 128


def build_nc(loop_n=1):
    nc = bacc.Bacc("TRN2", target_bir_lowering=False, debug=False, num_devices=1)

    xq = nc.dram_tensor("xq", [P, 8, S], BF16, kind="ExternalInput")
    xk = nc.dram_tensor("xk", [P, 8, S], BF16, kind="ExternalInput")
    xv = nc.dram_tensor("xv", [P, 8, S], BF16, kind="ExternalInput")
    wq = nc.dram_tensor("wq", [P, 8, DG], BF16, kind="ExternalInput")
    wk = nc.dram_tensor("wk", [P, 8, DG], BF16, kind="ExternalInput")
    wv = nc.dram_tensor("wv", [P, 8, DG], BF16, kind="ExternalInput")
    wo = nc.dram_tensor("wo", [P, 4, D], BF16, kind="ExternalInput")
    bq = nc.dram_tensor("bq", [P, 4], F32, kind="ExternalInput")
    cst = nc.dram_tensor("cst", [P, 64], BF16, kind="ExternalInput")  # ones
    out = nc.dram_tensor("out", [S, D], F32, kind="ExternalOutput")
    out_r = out.ap().rearrange("(st p) o -> p st o", p=P)  # [128, 16, 1024]

    import contextlib

    with tile.TileContext(nc) as tc:
        loop_cm = tc.For_i(0, loop_n, 1) if loop_n > 1 else contextlib.nullcontext()
        with loop_cm:
            _emit_body(nc, tc, locals_in := dict(
                xq=xq, xk=xk, xv=xv, wq=wq, wk=wk, wv=wv, wo=wo, bq=bq,
                cst=cst, out_r=out_r))
    nc.compile()
    return nc


def _emit_body(nc, tc, t_):
    xq, xk, xv = t_["xq"], t_["xk"], t_["xv"]
    wq, wk, wv, wo, bq, cst = t_["wq"], t_["wk"], t_["wv"], t_["wo"], t_["bq"], t_["cst"]
    out_r = t_["out_r"]
    if True:
        consts = tc.alloc_tile_pool(name="consts", bufs=1)
        qkv = tc.alloc_tile_pool(name="qkv", bufs=1)

        ones = consts.tile([P, 64], BF16, tag="ones")
        nc.sync.dma_start(ones[:], cst.ap())
        bq_sb = consts.tile([P, 4], F32, tag="bq")
        nc.sync.dma_start(bq_sb[:], bq.ap())
        wq_sb = consts.tile([P, 8, DG], BF16, tag="wq")
        nc.sync.dma_start(wq_sb[:], wq.ap())
        wk_sb = consts.tile([P, 8, DG], BF16, tag="wk")
        nc.sync.dma_start(wk_sb[:], wk.ap())
        wv_sb = consts.tile([P, 8, DG], BF16, tag="wv")
        nc.sync.dma_start(wv_sb[:], wv.ap())
        wo_sb = consts.tile([P, 4, D], BF16, tag="wo")
        nc.sync.dma_start(wo_sb[:], wo.ap())

        qT = qkv.tile([P, 4, S], BF16, tag="qT")   # [dims 128 (pair), pair, s]
        kT = qkv.tile([P, 4, S], BF16, tag="kT")
        v_sb = qkv.tile([P, KT, DG], BF16, tag="v")  # [k rows, ktile, dims]

        # ---- Phase 1: projections ----
        with tc.tile_pool(name="xs", bufs=1) as xs, \
             tc.tile_pool(name="psp", bufs=4, space="PSUM") as psp:
            xq_sb = xs.tile([P, 8, S], BF16, tag="xq")
            nc.sync.dma_start(xq_sb[:], xq.ap())
            xk_sb = xs.tile([P, 8, S], BF16, tag="xk")
            nc.sync.dma_start(xk_sb[:], xk.ap())
            xv_sb = xs.tile([P, 8, S], BF16, tag="xv")
            nc.sync.dma_start(xv_sb[:], xv.ap())

            for t in range(4):      # output-dim tile (= head pair)
                for c in range(QC):
                    psq = psp.tile([P, 512], F32, tag="ps")
                    for k in range(8):
                        nc.tensor.matmul(psq[:], lhsT=wq_sb[:, k, ts(t, P)],
                                         rhs=xq_sb[:, k, ts(c, 512)],
                                         start=(k == 0), stop=(k == 7))
                    nc.vector.tensor_scalar_add(qT[:, t, ts(c, 512)], psq[:],
                                                bq_sb[:, t:t + 1])
                    psk = psp.tile([P, 512], F32, tag="ps")
                    for k in range(8):
                        nc.tensor.matmul(psk[:], lhsT=wk_sb[:, k, ts(t, P)],
                                         rhs=xk_sb[:, k, ts(c, 512)],
                                         start=(k == 0), stop=(k == 7))
                    nc.vector.tensor_copy(kT[:, t, ts(c, 512)], psk[:])
            for st in range(KT):
                psv = psp.tile([P, 512], F32, tag="ps")
                for k in range(8):
                    nc.tensor.matmul(psv[:], lhsT=xv_sb[:, k, ts(st, P)],
                                     rhs=wv_sb[:, k, :],
                                     start=(k == 0), stop=(k == 7))
                nc.vector.tensor_copy(v_sb[:, st, :], psv[:])

        # ---- Phase 2: attention ----
        uT = qkv.tile([P, 4, S], BF16, tag="uT")
        with tc.tile_pool(name="exps", bufs=2) as exps, \
             tc.tile_pool(name="rn", bufs=2) as rn, \
             tc.tile_pool(name="ps_s", bufs=3, space="PSUM") as ps_s, \
             tc.tile_pool(name="ps_u", bufs=1, space="PSUM") as ps_u, \
             tc.tile_pool(name="ps_r", bufs=1, space="PSUM") as ps_r:
            for p in range(HP):
                for c in range(QC):
                    expA = exps.tile([P, KT, 512], BF16, tag="expA")
                    expB = exps.tile([P, KT, 512], BF16, tag="expB")
                    psU = ps_u.tile([P, 512], F32, tag="u")
                    psR = ps_r.tile([33, 512], F32, tag="r")
                    for g in range(KT // 2):
                        psS0 = ps_s.tile([P, 2, 512], F32, tag="s")
                        psS1 = ps_s.tile([P, 2, 512], F32, tag="s")
                        for i in range(2):
                            k = 2 * g + i
                            nc.tensor.matmul(psS0[:, i, :],
                                             lhsT=kT[0:64, p, ts(k, P)],
                                             rhs=qT[0:64, p, ts(c, 512)],
                                             start=True, stop=True)
                            nc.tensor.matmul(psS1[:, i, :],
                                             lhsT=kT[64:128, p, ts(k, P)],
                                             rhs=qT[64:128, p, ts(c, 512)],
                                             start=True, stop=True)
                        nc.scalar.activation(expA[:, ts(g, 2), :], psS0[:],
                                             AF.Exp, scale=0.125)
                        nc.scalar.activation(expB[:, ts(g, 2), :], psS1[:],
                                             AF.Exp, scale=0.125)
                    for k in range(KT):
                        fl, ll = (k == 0), (k == KT - 1)
                        nc.tensor.matmul(psU[0:64, :],
                                         lhsT=v_sb[:, k, ds(P * p, 64)],
                                         rhs=expA[:, k, :], start=fl, stop=ll,
                                         tile_position=(0, 0),
                                         skip_group_check=True)
                        nc.tensor.matmul(psU[64:128, :],
                                         lhsT=v_sb[:, k, ds(P * p + 64, 64)],
                                         rhs=expB[:, k, :], start=fl, stop=ll,
                                         tile_position=(0, 64),
                                         skip_group_check=True)
                        nc.tensor.matmul(psR[0:1, :], lhsT=ones[:, 0:1],
                                         rhs=expA[:, k, :], start=fl, stop=ll,
                                         tile_position=(0, 0),
                                         skip_group_check=True)
                        nc.tensor.matmul(psR[32:33, :], lhsT=ones[:, 0:1],
                                         rhs=expB[:, k, :], start=fl, stop=ll,
                                         tile_position=(0, 32),
                                         skip_group_check=True)
                    # normalize: recip(r) -> broadcast (K=1 matmul) -> multiply
                    rsb = rn.tile([33, 512], BF16, tag="rsb")
                    with nc.allow_low_precision(reason="softmax denom recip in bf16"):
                        nc.vector.reciprocal(rsb[0:1, :], psR[0:1, :])
                        nc.vector.reciprocal(rsb[32:33, :], psR[32:33, :])
                    psBc = ps_s.tile([P, 2, 512], F32, tag="s")
                    nc.tensor.matmul(psBc[0:64, 0, :], lhsT=ones[0:1, :],
                                     rhs=rsb[0:1, :], start=True, stop=True,
                                     tile_position=(0, 0),
                                     skip_group_check=True)
                    nc.tensor.matmul(psBc[64:128, 0, :], lhsT=ones[32:33, :],
                                     rhs=rsb[32:33, :], start=True, stop=True,
                                     tile_position=(32, 64),
                                     skip_group_check=True)
                    bcsb = rn.tile([P, 512], BF16, tag="bcsb")
                    nc.vector.tensor_copy(bcsb[:], psBc[:, 0, :])
                    nc.vector.tensor_mul(uT[:, p, ts(c, 512)], psU[:], bcsb[:])

        # ---- Phase 3: output projection ----
        with tc.tile_pool(name="ys", bufs=3) as ys, \
             tc.tile_pool(name="ps_y", bufs=4, space="PSUM") as ps_y:
            for st in range(KT):
                ysb = ys.tile([P, D], F32, tag="y")
                for oc in range(2):
                    psy = ps_y.tile([P, 512], F32, tag="y")
                    for p in range(HP):
                        nc.tensor.matmul(psy[:], lhsT=uT[:, p, ts(st, P)],
                                         rhs=wo_sb[:, p, ts(oc, 512)],
                                         start=(p == 0), stop=(p == HP - 1))
                    nc.vector.tensor_copy(ysb[:, ts(oc, 512)], psy[:])
                nc.sync.dma_start(out_r[:, st, :], ysb[:])

        qkv.release()
        consts.release()


_NC = None


def _get_nc():
    global _NC
    if _NC is None:
        _NC = build_nc()
    return _NC


def _to3(x, inner):
    # [128*n, inner] -> [128, n, inner] partition-major
    n = x.shape[0] // P
    return np.ascontiguousarray(x.reshape(n, P, inner).transpose(1, 0, 2))


def make_in_maps(query, key, value, Wq, bq, Wk, bk, Wv, bv, Wo, bo):
    ones = np.ones((P, 64), bf16)
    in_maps = []
    for core in range(8):
        b, g = core // 2, core % 2
        sel = slice(g * DG, (g + 1) * DG)
        m = {
            "xq": _to3(np.ascontiguousarray(query[b].T).astype(bf16), S),
            "xk": _to3(np.ascontiguousarray(key[b].T).astype(bf16), S),
            "xv": _to3(np.ascontiguousarray(value[b].T).astype(bf16), S),
            "wq": _to3(np.ascontiguousarray(Wq[sel].T).astype(bf16), DG),
            "wk": _to3(np.ascontiguousarray(Wk[sel].T).astype(bf16), DG),
            "wv": _to3(np.ascontiguousarray(Wv[sel].T).astype(bf16), DG),
            "wo": _to3(np.ascontiguousarray(Wo[:, sel].T).astype(bf16), D),
            "bq": np.ascontiguousarray(bq[sel].reshape(4, P).T).astype(np.float32),
            "cst": ones,
        }
        in_maps.append(m)
    return in_maps


def combine(results, Wo, bv, bo):
    bo_eff = (bo + bv @ Wo.T).astype(np.float32)
    out = np.empty((B, S, D), np.float32)
    for b in range(B):
        out[b] = results[2 * b]["out"] + results[2 * b + 1]["out"] + bo_eff
    return out


def kernel(query, key, value, Wq, bq, Wk, bk, Wv, bv, Wo, bo):
    nc = _get_nc()
    in_maps = make_in_maps(query, key, value, Wq, bq, Wk, bk, Wv, bv, Wo, bo)
    res = run_bass_kernel_spmd(nc, in_maps, core_ids=list(range(8)))
    return combine(res.results, Wo, bv, bo)
